# revision 25
# baseline (speedup 1.0000x reference)
"""DeepseekMoE kernel for 8 Trainium2 NeuronCores.

Strategy (expert-parallel + data-parallel shared experts):
  - Host computes the router (gate matmul, softmax, top-2) in numpy and
    gathers each expert's tokens (classic MoE dispatch, done host-side as
    part of sharding).
  - Core c runs routed expert c's FFN over its gathered tokens (padded to
    a common Cpad so all 8 cores run the same SPMD program), scaling the
    output by the combine weights on-device (DVE).
  - Shared experts' weights are replicated; each core runs them over a
    distinct 512-token slice of the batch (data-parallel).
  - All matmuls run in bf16 (1 cycle/row on the PE vs 4 for fp32) with
    fp32 PSUM accumulation; GELU (exact/erf) on the ACT engine reading
    PSUM directly.
  - Layout is fully transposed (features on partitions, tokens on the
    free dim) so the two FFN matmuls chain with no on-chip transposes.
    Host pre-packs every operand into [128, *] row-major blocks so each
    DMA is a contiguous >=512 KB transfer (HWDGE generation overhead is
    ~625 ns/DMA, so small DMAs cap effective HBM bandwidth).
  - The f-loop is software-pipelined (lookahead 2) across chunks and
    phases so the PE never stalls on ACT; output DMAs ride the SWDGE
    (gpsimd) path so they are not head-of-line blocked behind the
    input preload on the HWDGE queues.
  - Host scatters per-expert outputs back (each token appears in exactly
    K=2 experts) and adds the (zero, but handled exactly) output biases.
"""

import numpy as np
import ml_dtypes

import concourse.bass as bass
import concourse.tile as tile
import concourse.mybir as mybir
from concourse import bacc
from concourse.bass_utils import run_bass_kernel_spmd

B, S, D, F, E, NS, K = 2, 2048, 512, 2048, 8, 2, 2
T = B * S
N_CORES = 8
TS = T // N_CORES          # shared-expert tokens per core
FS = NS * F                # concatenated shared FFN width
CHUNK = 512                # token chunk (= max fp32 PSUM bank free dim)
KD = D // 128              # 4  k-tiles over D
FR = F // 128              # 16 f-tiles routed
FShared = FS // 128        # 32 f-tiles shared
DD = D // 128              # 4  output d-tiles
WG = 4                     # f-tiles per w-DMA group (512 KB transfers)

BF16 = mybir.dt.bfloat16
F32 = mybir.dt.float32
np_bf16 = ml_dtypes.bfloat16

_GELU = mybir.ActivationFunctionType.Gelu

_cache: dict = {}


def _routed_sizes(cpad):
    """Token-chunk sizes for the routed phase: a small first chunk (fast PE
    start — less DMA to wait for), 512s in the middle, and a smallish final
    chunk (short drain tail). No chunk below 256 — small-N matmuls go
    LDWEIGHTS-bound on real hardware."""
    if cpad <= CHUNK:
        return [cpad]
    sizes, rem = [256], cpad - 256
    while rem > CHUNK:
        take = CHUNK if rem - CHUNK >= 256 or rem == 2 * CHUNK else rem - 384
        sizes.append(take)
        rem -= take
    sizes.append(rem)
    return sizes


def _chunk_offsets(total, sizes=None):
    """(start, size) pairs; default uniform CHUNK split."""
    if sizes is None:
        sizes = [min(CHUNK, total - c0) for c0 in range(0, total, CHUNK)]
    out, c0 = [], 0
    for s in sizes:
        out.append((c0, s))
        c0 += s
    return out


def _build(cpad: int):
    nc = bacc.Bacc("TRN2", debug=False)

    xg = nc.dram_tensor("xg", [128, KD * cpad], BF16, kind="ExternalInput")
    cwb = nc.dram_tensor("cwb", [128, cpad], F32, kind="ExternalInput")
    rw1t = nc.dram_tensor("rw1t", [128, KD * F], BF16, kind="ExternalInput")
    rw2t = nc.dram_tensor("rw2t", [128, FR * D], BF16, kind="ExternalInput")
    rb1 = nc.dram_tensor("rb1", [128, FR], F32, kind="ExternalInput")
    xs = nc.dram_tensor("xs", [128, KD * TS], BF16, kind="ExternalInput")
    sw1t = nc.dram_tensor("sw1t", [128, KD * FS], BF16, kind="ExternalInput")
    sw2t = nc.dram_tensor("sw2t", [128, FShared * D], BF16, kind="ExternalInput")
    sb1 = nc.dram_tensor("sb1", [128, FShared], F32, kind="ExternalInput")
    yr = nc.dram_tensor("yr", [D, cpad], F32, kind="ExternalOutput")
    ys = nc.dram_tensor("ys", [D, TS], F32, kind="ExternalOutput")

    with tile.TileContext(nc) as tc:
        with (
            tc.tile_pool(name="wts", bufs=1) as wts,
            tc.tile_pool(name="acts", bufs=1) as acts,
            tc.tile_pool(name="hp", bufs=4) as hp,
            tc.tile_pool(name="op", bufs=3) as op,
            tc.tile_pool(name="ps1", bufs=4, space="PSUM") as ps1,
            tc.tile_pool(name="ps2", bufs=1, space="PSUM") as ps2,
        ):
            # ---- resident SBUF images of all inputs ----
            xg_sb = acts.tile([128, KD * cpad], BF16, name="xg_sb")
            rw1_sb = wts.tile([128, KD * F], BF16, name="rw1_sb")
            rw2_sb = wts.tile([128, FR * D], BF16, name="rw2_sb")
            rb1_sb = wts.tile([128, FR], F32, name="rb1_sb")
            cw_sb = acts.tile([128, cpad], F32, name="cw_sb")
            xs_sb = acts.tile([128, KD * TS], BF16, name="xs_sb")
            sw1_sb = wts.tile([128, KD * FS], BF16, name="sw1_sb")
            sw2_sb = wts.tile([128, FShared * D], BF16, name="sw2_sb")
            sb1_sb = wts.tile([128, FShared], F32, name="sb1_sb")

            def col_dma(dst, src, lo, hi):
                nc.sync.dma_start(dst[:, lo:hi], src.ap()[:, lo:hi])

            def w1_group_dma(dst, src, f_lo, f_hi):
                # f-columns [f_lo*128, f_hi*128) for every k-block
                d4 = dst.rearrange("p (k f) -> p k f", k=KD)
                s4 = src.ap().rearrange("p (k f) -> p k f", k=KD)
                nc.sync.dma_start(d4[:, :, f_lo * 128:f_hi * 128],
                                  s4[:, :, f_lo * 128:f_hi * 128])

            # consumption-ordered preload (HWDGE)
            chunks_r = _chunk_offsets(cpad, _routed_sizes(cpad))
            c0, cs = chunks_r[0]
            xoff = [0]
            for _, s in chunks_r:
                xoff.append(xoff[-1] + KD * s)
            col_dma(xg_sb, xg, 0, xoff[1])                      # chunk 0 tokens
            w1_group_dma(rw1_sb, rw1t, 0, 2)                    # rw1 f0..f1
            nc.sync.dma_start(rb1_sb[:], rb1.ap())
            col_dma(rw2_sb, rw2t, 0, WG * D)                    # rw2 f0..f3
            w1_group_dma(rw1_sb, rw1t, 2, 4)
            for g in range(1, FR // WG):
                w1_group_dma(rw1_sb, rw1t, g * WG, (g + 1) * WG)
                col_dma(rw2_sb, rw2t, g * WG * D, (g + 1) * WG * D)
            col_dma(xg_sb, xg, xoff[1], xoff[-1])               # remaining tokens
            nc.sync.dma_start(cw_sb[:], cwb.ap())
            nc.sync.dma_start(xs_sb[:], xs.ap())
            nc.sync.dma_start(sb1_sb[:], sb1.ap())
            for g in range(FShared // (2 * WG)):                # 1 MB transfers
                w1_group_dma(sw1_sb, sw1t, g * 2 * WG, (g + 1) * 2 * WG)
                col_dma(sw2_sb, sw2t, g * 2 * WG * D, (g + 1) * 2 * WG * D)

            # ---- chunk descriptors: small routed chunk first (fast start),
            # shared phase in the middle, small routed chunk last (short tail) ----
            def r_chunk(i, c0, cs):
                return dict(
                    cs=cs, c0=c0, nf=FR, cw=True, y=yr, b1=rb1_sb,
                    x=lambda k, o=xoff[i], cs=cs: xg_sb[:, o + k * cs:o + (k + 1) * cs],
                    w1=lambda k, f: rw1_sb[:, k * F + f * 128:k * F + (f + 1) * 128],
                    w2=lambda f, d: rw2_sb[:, f * D + d * 128:f * D + (d + 1) * 128],
                )

            def s_chunk(c0, cs):
                return dict(
                    cs=cs, c0=c0, nf=FShared, cw=False, y=ys, b1=sb1_sb,
                    x=lambda k, c0=c0, cs=cs: xs_sb[:, KD * c0 + k * cs:KD * c0 + (k + 1) * cs],
                    w1=lambda k, f: sw1_sb[:, k * FS + f * 128:k * FS + (f + 1) * 128],
                    w2=lambda f, d: sw2_sb[:, f * D + d * 128:f * D + (d + 1) * 128],
                )

            routed = [r_chunk(i, c0, cs) for i, (c0, cs) in enumerate(chunks_r)]
            shared = [s_chunk(c0, cs) for c0, cs in _chunk_offsets(TS)]
            chunks = routed[:-1] + shared + routed[-1:]
            steps = [(ch, f) for ch in chunks for f in range(ch["nf"])]

            # ---- software-pipelined emission: PE issues the f-tile's
            # first-layer matmuls LOOKAHEAD steps ahead of the second-layer
            # matmuls that consume the GELU output ----
            LOOKAHEAD = 2
            h_tiles: dict = {}
            po_tiles: dict = {}
            for i in range(len(steps) + LOOKAHEAD):
                if i < len(steps):
                    ch, f = steps[i]
                    cs = ch["cs"]
                    p1 = ps1.tile([128, cs], F32, name="p1")
                    for k in range(KD):
                        nc.tensor.matmul(
                            p1[:], ch["w1"](k, f), ch["x"](k),
                            start=(k == 0), stop=(k == KD - 1),
                        )
                    h = hp.tile([128, cs], BF16, name="h")
                    nc.scalar.activation(h[:], p1[:], _GELU, bias=ch["b1"][:, f:f + 1])
                    h_tiles[i] = h
                j = i - LOOKAHEAD
                if j >= 0:
                    ch, f = steps[j]
                    cs, c0 = ch["cs"], ch["c0"]
                    if f == 0:
                        po_tiles[id(ch)] = [
                            ps2.tile([128, cs], F32, tag=f"o{d}", name=f"po{d}")
                            for d in range(DD)
                        ]
                    po = po_tiles[id(ch)]
                    h = h_tiles.pop(j)
                    for d in range(DD):
                        nc.tensor.matmul(
                            po[d][:], ch["w2"](f, d), h[:],
                            start=(f == 0), stop=(f == ch["nf"] - 1),
                        )
                    if f == ch["nf"] - 1:
                        o = op.tile([128, DD * cs], F32, name="o")
                        for d in range(DD):
                            if ch["cw"]:
                                nc.vector.tensor_mul(
                                    o[:, d * cs:(d + 1) * cs], po[d][:],
                                    cw_sb[:, c0:c0 + cs])
                            else:
                                nc.vector.tensor_copy(
                                    o[:, d * cs:(d + 1) * cs], po[d][:])
                        # one wide DMA per chunk on the SWDGE path: separate
                        # FIFO from the input preload (no head-of-line block),
                        # and one generation overhead instead of four. The
                        # final chunk rides HWDGE (lower latency; preload is
                        # long finished) to shorten the kernel tail.
                        ydst = ch["y"].ap().rearrange(
                            "(dd p) c -> p dd c", p=128)[:, :, c0:c0 + cs]
                        ysrc = o.rearrange("p (dd c) -> p dd c", dd=DD)
                        if ch is chunks[-1]:
                            nc.sync.dma_start(ydst, ysrc)
                        else:
                            nc.gpsimd.dma_start(ydst, ysrc)
                        del po_tiles[id(ch)]

    nc.compile()
    return nc


def _pack_k_blocks(a2d):
    """[K*128, N] -> [128, K*N] with k-blocks along the free dim."""
    k = a2d.shape[0] // 128
    return np.ascontiguousarray(
        a2d.reshape(k, 128, -1).transpose(1, 0, 2).reshape(128, -1))


def _pack_chunked(xT, total, sizes=None):
    """[D, total] -> [128, KD*total] grouped chunk-major: for each chunk c,
    the KD k-blocks of that chunk's columns are laid out consecutively."""
    parts = []
    for c0, cs in _chunk_offsets(total, sizes):
        blk = xT[:, c0:c0 + cs]                      # [D, cs]
        parts.append(blk.reshape(KD, 128, cs).transpose(1, 0, 2).reshape(128, -1))
    return np.ascontiguousarray(np.concatenate(parts, axis=1))


def kernel(x, gate_w, gate_b, sw1, sb1, sw2, sb2, rw1, rb1, rw2, rb2):
    x = np.asarray(x, np.float32)
    gate_w = np.asarray(gate_w, np.float32)
    gate_b = np.asarray(gate_b, np.float32)
    sw1 = np.asarray(sw1, np.float32)
    sb1 = np.asarray(sb1, np.float32)
    sw2 = np.asarray(sw2, np.float32)
    sb2 = np.asarray(sb2, np.float32)
    rw1 = np.asarray(rw1, np.float32)
    rb1 = np.asarray(rb1, np.float32)
    rw2 = np.asarray(rw2, np.float32)
    rb2 = np.asarray(rb2, np.float32)

    t = x.reshape(T, D)

    # ---- router on host (part of the dispatch/sharding step) ----
    logits = t @ gate_w.T + gate_b
    m = logits.max(axis=1, keepdims=True)
    ex = np.exp(logits - m)
    probs = ex / ex.sum(axis=1, keepdims=True)
    top_i = np.argpartition(-probs, K - 1, axis=1)[:, :K]          # [T, K]

    sel = np.zeros((T, E), bool)
    sel[np.arange(T)[:, None], top_i] = True
    idxs = [np.nonzero(sel[:, e])[0] for e in range(E)]
    counts = np.array([len(i) for i in idxs])
    cpad = max(CHUNK, int(-(-counts.max() // 128) * 128))

    if cpad not in _cache:
        _cache[cpad] = _build(cpad)
    nc = _cache[cpad]

    # ---- shared-expert weights, concatenated over NS and packed ----
    sw1t = _pack_k_blocks(sw1.reshape(FS, D).T.astype(np_bf16))
    sw2t = _pack_k_blocks(sw2.transpose(0, 2, 1).reshape(FS, D).astype(np_bf16))
    sb1c = np.ascontiguousarray(sb1.reshape(FShared, 128).T)

    in_maps = []
    for c in range(N_CORES):
        idx = idxs[c]
        ce = len(idx)
        xgT = np.zeros((D, cpad), np_bf16)
        xgT[:, :ce] = t[idx].T.astype(np_bf16)
        cwb = np.zeros((128, cpad), np.float32)
        cwb[:, :ce] = probs[idx, c][None, :]
        in_maps.append({
            "xg": _pack_chunked(xgT, cpad, _routed_sizes(cpad)),
            "cwb": cwb,
            "rw1t": _pack_k_blocks(rw1[c].T.astype(np_bf16)),
            "rw2t": _pack_k_blocks(rw2[c].T.astype(np_bf16)),
            "rb1": np.ascontiguousarray(rb1[c].reshape(FR, 128).T),
            "xs": _pack_chunked(
                np.ascontiguousarray(t[c * TS:(c + 1) * TS].T.astype(np_bf16)), TS),
            "sw1t": sw1t,
            "sw2t": sw2t,
            "sb1": sb1c,
        })

    res = run_bass_kernel_spmd(nc, in_maps, core_ids=list(range(N_CORES)))

    # ---- combine on host ----
    out = np.empty((T, D), np.float32)
    for c in range(N_CORES):
        out[c * TS:(c + 1) * TS] = res.results[c]["ys"].T
    for c in range(N_CORES):
        idx = idxs[c]
        out[idx] += res.results[c]["yr"][:, :len(idx)].T

    # output biases (zero in the spec, handled exactly anyway)
    if sb2.any() or rb2.any():
        cw = np.zeros((T, E), np.float32)
        np.add.at(cw, (np.arange(T)[:, None], top_i),
                  np.take_along_axis(probs, top_i, axis=1))
        out += sb2.sum(axis=0)[None, :] + cw @ rb2

    return out.reshape(B, S, D)
